# revision 14
# baseline (speedup 1.0000x reference)
"""Trainium2 Bass kernel for GQA multi-head attention with RoPE.

Sharding: tensor-parallel over heads. Core c owns q-heads 4c..4c+3 and
kv-head c. Each core computes its QKV projection slice, RoPE, causal
attention for its heads, and a partial output projection
(attn_out_local @ Wo[:, local].T). The host sums the 8 partial y's.

Device layouts (per core):
  xT      [C, B*T]   x transposed (replicated)
  wcatT   [C, 384]   [Wq_loc | Wk_loc | Wv_loc].T
  qT/kT   [d, tok]   head-dim on partitions ("T-layout")
  scores  S^T [tk partitions, tq free] so no transposes are needed:
          exp(S^T) feeds P@V directly as the moving operand with
          v_aug = [v | ones] stationary; the ones row yields softmax
          row-sums in partition 64 of the PV accumulator.
  y       [B*T, C]   partial; host adds the 8 partials + bo.

All matmul operands are float32r (fp32 with 11-bit mantissa, full PE
rate at N>=256); PSUM accumulation is fp32.
"""

import sys

sys.path.insert(0, "/opt/trn_rl_repo")

import numpy as np

import bass_rust
import concourse.bass as bass
import concourse.tile as tile
from concourse import mybir
from concourse.bass_utils import run_bass_kernel_spmd

B, T, C = 2, 2048, 2048
H, KVH, D = 32, 8, 64
NTOK = B * T                 # 4096
HPC = H // 8                 # 4 q heads per core
QL = HPC * D                 # 256 local q dims
KC = C // 128                # 16 contraction chunks
SCALE = float(D) ** -0.5

F32 = mybir.dt.float32
F32R = mybir.dt.float32r
AF = mybir.ActivationFunctionType

_NC_CACHE = {}


def _split_waits(nc, limit=1):
    """Walrus in this toolchain allows only one sync-wait per instruction.

    Tile emits instructions with several sem waits (drain/barrier, phase
    boundaries). Hoist the excess onto same-engine NoOps inserted right
    before the instruction — program order on the engine queue preserves
    the wait semantics.
    """
    ctr = 0
    for f in nc.m.functions:
        for blk in f.blocks:
            out = []
            changed = False
            for inst in list(blk.instructions):
                si = inst.sync_info
                if si is not None and len(si.on_wait) > limit:
                    waits = list(si.on_wait)
                    keep, excess = waits[:limit], waits[limit:]
                    for i in range(0, len(excess), limit):
                        ctr += 1
                        nop = mybir.InstNoOp(
                            name=f"I-wsplit-{ctr}", ins=[], outs=[]
                        )
                        nop.engine = inst.engine
                        nop.sync_info = bass_rust.SyncInfo(
                            on_wait=excess[i : i + limit], on_update=[]
                        )
                        out.append(nop)
                        changed = True
                    inst.sync_info = bass_rust.SyncInfo(
                        on_wait=keep, on_update=list(si.on_update)
                    )
                out.append(inst)
            if changed:
                blk.instructions = out
    return ctr


def build_nc():
    nc = bass.Bass(trn_type="TRN2")

    xT_d = nc.dram_tensor("xT", [C, NTOK], F32R, kind="ExternalInput")
    wcat_d = nc.dram_tensor("wcatT", [C, 384], F32R, kind="ExternalInput")
    bias_d = nc.dram_tensor("bqkv", [128, 3], F32, kind="ExternalInput")
    wo_d = nc.dram_tensor("woR", [QL, C], F32R, kind="ExternalInput")
    cos_d = nc.dram_tensor("cos2", [128, NTOK], F32R, kind="ExternalInput")
    sin_d = nc.dram_tensor("sin2", [128, NTOK], F32R, kind="ExternalInput")
    pmat_d = nc.dram_tensor("pmat", [128, 128], F32R, kind="ExternalInput")
    dmask_d = nc.dram_tensor("dmask", [128, 128], F32R, kind="ExternalInput")
    ident_d = nc.dram_tensor("ident2", [128, 64], F32R, kind="ExternalInput")
    vones_d = nc.dram_tensor(
        "vones", [128, 2 * (T // 128)], F32R, kind="ExternalInput"
    )
    y_d = nc.dram_tensor("y", [NTOK, C], F32, kind="ExternalOutput")

    with tile.TileContext(nc) as tc:
        with (
            tc.tile_pool(name="consts", bufs=1) as consts,
            tc.tile_pool(name="xs", bufs=3) as xs_pool,
            tc.tile_pool(name="acts", bufs=1) as acts,
            tc.tile_pool(name="big", bufs=2) as big,
            tc.tile_pool(name="tmp", bufs=1) as tmp_pool,
            tc.tile_pool(name="es", bufs=2) as es_pool,
            tc.tile_pool(name="rows", bufs=2) as rows,
            tc.tile_pool(name="ibc", bufs=2) as ibc_pool,
            tc.tile_pool(name="onorm", bufs=2) as on_pool,
            tc.tile_pool(name="ystage", bufs=2) as y_pool,
            tc.tile_pool(name="psA", bufs=2, space="PSUM") as psA,
            tc.tile_pool(name="psB", bufs=2, space="PSUM") as psB,
        ):
            # ---- constants ----
            wcat_sb = consts.tile([128, KC, 384], F32R, tag="wcat")
            for k in range(KC):
                nc.sync.dma_start(
                    out=wcat_sb[:, k, :], in_=wcat_d[128 * k : 128 * (k + 1), :]
                )
            bias_sb = consts.tile([128, 3], F32, tag="bias")
            nc.sync.dma_start(out=bias_sb, in_=bias_d[:, :])
            wo_sb = consts.tile([128, 2, C], F32R, tag="wo")
            for k in range(2):
                nc.sync.dma_start(
                    out=wo_sb[:, k, :], in_=wo_d[128 * k : 128 * (k + 1), :]
                )
            pmat_sb = consts.tile([128, 128], F32R, tag="pmat")
            nc.sync.dma_start(out=pmat_sb, in_=pmat_d[:, :])
            dmask_sb = consts.tile([128, 128], F32R, tag="dmask")
            nc.sync.dma_start(out=dmask_sb, in_=dmask_d[:, :])
            ident_sb = consts.tile([128, 64], F32R, tag="ident")
            nc.sync.dma_start(out=ident_sb, in_=ident_d[:, :])

            cos_sb = big.tile([128, NTOK], F32R, tag="big")
            nc.sync.dma_start(out=cos_sb, in_=cos_d[:, :])
            sin_sb = big.tile([128, NTOK], F32R, tag="big")
            nc.sync.dma_start(out=sin_sb, in_=sin_d[:, :])

            qT01 = acts.tile([128, NTOK], F32R, tag="qT01")
            qT23 = acts.tile([128, NTOK], F32R, tag="qT23")
            kvT = acts.tile([128, NTOK], F32R, tag="kvT")
            ktdup = acts.tile([128, NTOK], F32R, tag="ktdup")
            vaug = acts.tile([128, 2 * (T // 128), 65], F32R, tag="vaug")

            # ---- phase 1: QKV projection (qT/kT/vT layout) ----
            qkv_dst = [qT01, qT23, kvT]
            for n in range(NTOK // 512):
                ps0 = psA.tile([128, 512], F32, tag="a")
                ps1 = psA.tile([128, 512], F32, tag="a")
                ps2 = psB.tile([128, 512], F32, tag="o")
                ps = [ps0, ps1, ps2]
                for k in range(KC):
                    xt = xs_pool.tile([128, 512], F32R, tag="xs")
                    nc.sync.dma_start(
                        out=xt,
                        in_=xT_d[128 * k : 128 * (k + 1), 512 * n : 512 * (n + 1)],
                    )
                    for m in range(3):
                        nc.tensor.matmul(
                            ps[m],
                            wcat_sb[:, k, 128 * m : 128 * (m + 1)],
                            xt,
                            start=(k == 0),
                            stop=(k == KC - 1),
                        )
                for m in range(3):
                    nc.scalar.activation(
                        out=qkv_dst[m][:, 512 * n : 512 * (n + 1)],
                        in_=ps[m],
                        func=AF.Identity,
                        bias=bias_sb[:, m : m + 1],
                        scale=1.0,
                    )

            # ---- phase 2: RoPE on q (all 128 rows) and k (rows 0:64) ----
            def rope(dst, rows_n):
                # dst[0:rows_n] = dst*cos + (P @ dst)*sin, in 2048-halves
                for half in range(2):
                    sl = slice(2048 * half, 2048 * (half + 1))
                    tmp = tmp_pool.tile([128, 2048], F32, tag="tmp")
                    for cc in range(2):
                        rps = psA.tile([128, 1024], F32, tag="a")
                        for c5 in range(2):
                            o = 2048 * half + 1024 * cc + 512 * c5
                            nc.tensor.matmul(
                                rps[: rows_n, 512 * c5 : 512 * (c5 + 1)],
                                pmat_sb[:rows_n, :rows_n],
                                dst[:rows_n, o : o + 512],
                                start=True,
                                stop=True,
                            )
                        nc.vector.tensor_mul(
                            tmp[:rows_n, 1024 * cc : 1024 * (cc + 1)],
                            rps[:rows_n, :],
                            sin_sb[:rows_n, 2048 * half + 1024 * cc :
                                   2048 * half + 1024 * (cc + 1)],
                        )
                    nc.vector.tensor_mul(
                        dst[:rows_n, sl], dst[:rows_n, sl], cos_sb[:rows_n, sl]
                    )
                    nc.vector.tensor_add(
                        dst[:rows_n, sl], dst[:rows_n, sl], tmp[:rows_n, :]
                    )

            rope(qT01, 128)
            rope(qT23, 128)
            rope(kvT, 64)  # k rows only; v rows untouched

            # duplicate roped kT into partitions 64:128 for odd heads
            nc.sync.dma_start(out=ktdup[64:128, :], in_=kvT[0:64, :])

            # ---- v transpose into v_aug = [v | 1] (natural [tk, d] layout) ----
            nc.sync.dma_start(
                out=vaug[:, :, 64:65], in_=vones_d[:, :].unsqueeze(2)
            )
            for b2 in range(B):
                for jt in range(T // 128):
                    vps = psA.tile([128, 64], F32R, tag="a")
                    nc.tensor.transpose(
                        vps,
                        kvT[64:128, T * b2 + 128 * jt : T * b2 + 128 * (jt + 1)],
                        ident_sb[64:128, :],
                    )
                    nc.vector.tensor_copy(
                        vaug[:, b2 * (T // 128) + jt, 0:64], vps
                    )

            # ---- phase 3: attention, per (batch, head, tq-half) ----
            OT0 = big.tile([128, NTOK], F32R, tag="big")
            OT1 = big.tile([128, NTOK], F32R, tag="big")
            OT = [OT0, OT1]
            NJ = T // 128  # 16
            for b2 in range(B):
                for hq in range(HPC):
                    qtile = qT01 if hq < 2 else qT23
                    qr = 64 * (hq % 2)
                    kt = kvT if hq % 2 == 0 else ktdup
                    kr = 0 if hq % 2 == 0 else 64
                    for v2 in range(2):
                        tq0 = 1024 * v2
                        jmax = 8 * v2 + 8
                        ops = psB.tile([65, 1024], F32, tag="o")
                        last_bank = [8 * v2 + 3, jmax - 1]
                        for j in range(jmax):
                            tqs = max(128 * j, tq0)
                            W = tq0 + 1024 - tqs
                            sps = psA.tile([128, 1024], F32, tag="a")
                            off = 0
                            while off < W:
                                nw = min(512, W - off)
                                nc.tensor.matmul(
                                    sps[:, off : off + nw],
                                    kt[kr : kr + 64,
                                       T * b2 + 128 * j : T * b2 + 128 * (j + 1)],
                                    qtile[qr : qr + 64,
                                          T * b2 + tqs + off :
                                          T * b2 + tqs + off + nw],
                                    start=True,
                                    stop=True,
                                )
                                off += nw
                            es = es_pool.tile([128, 1024], F32R, tag="es")
                            nc.scalar.activation(
                                out=es[:, :W], in_=sps[:, :W],
                                func=AF.Exp, scale=SCALE,
                            )
                            if 128 * j >= tq0:
                                nc.vector.tensor_mul(
                                    es[:, 0:128], es[:, 0:128], dmask_sb
                                )
                            off = 0
                            while off < W:
                                oc = tqs - tq0 + off
                                bank = oc // 512
                                nw = min(512 - oc % 512, W - off)
                                nc.tensor.matmul(
                                    ops[:, oc : oc + nw],
                                    vaug[:, b2 * NJ + j, :],
                                    es[:, off : off + nw],
                                    start=(j == 0),
                                    stop=(j == last_bank[bank]),
                                )
                                off += nw
                        # softmax denominators: 1/rowsum via exp(-ln(.))
                        lnr = rows.tile([1, 1024], F32, tag="lnr")
                        nc.scalar.activation(
                            out=lnr, in_=ops[64:65, :], func=AF.Ln
                        )
                        invr = rows.tile([1, 1024], F32, tag="invr")
                        nc.scalar.activation(
                            out=invr, in_=lnr, func=AF.Exp, scale=-1.0
                        )
                        scr = nc.dram_tensor(
                            f"scr_{b2}_{hq}_{v2}", [1, 1024], F32
                        )
                        nc.sync.dma_start(out=scr[:, :], in_=invr)
                        ibc = ibc_pool.tile([64, 1024], F32, tag="ibc")
                        nc.sync.dma_start(
                            out=ibc,
                            in_=scr[:, :].partition_broadcast(64).squeeze(1),
                        )
                        ot = OT[hq // 2]
                        tok0 = T * b2 + tq0
                        if hq % 2 == 0:
                            nc.vector.tensor_mul(
                                ot[0:64, tok0 : tok0 + 1024],
                                ops[0:64, :],
                                ibc,
                            )
                        else:
                            on = on_pool.tile([64, 1024], F32R, tag="on")
                            nc.vector.tensor_mul(on, ops[0:64, :], ibc)
                            nc.sync.dma_start(
                                out=ot[64:128, tok0 : tok0 + 1024], in_=on
                            )

            # ---- phase 4: partial output projection ----
            for mt in range(NTOK // 128):
                for ncc in range(C // 512):
                    pool = psA if ncc % 2 == 0 else psB
                    tag = "a" if ncc % 2 == 0 else "o"
                    yp = pool.tile([128, 512], F32, tag=tag)
                    for kq in range(2):
                        nc.tensor.matmul(
                            yp,
                            OT[kq][:, 128 * mt : 128 * (mt + 1)],
                            wo_sb[:, kq, 512 * ncc : 512 * (ncc + 1)],
                            start=(kq == 0),
                            stop=(kq == 1),
                        )
                    ys = y_pool.tile([128, 512], F32, tag="ys")
                    if ncc % 2 == 0:
                        nc.scalar.copy(ys, yp)
                    else:
                        nc.vector.tensor_copy(ys, yp)
                    nc.sync.dma_start(
                        out=y_d[128 * mt : 128 * (mt + 1),
                                512 * ncc : 512 * (ncc + 1)],
                        in_=ys,
                    )
    _split_waits(nc)
    return nc


def _host_inputs(x, sinusoidal_pos, Wq, bq, Wk, bk, Wv, bv, Wo):
    xT = np.ascontiguousarray(x.reshape(NTOK, C).T).astype(np.float32)

    sp = np.asarray(sinusoidal_pos, dtype=np.float32).reshape(T, D)
    cosd = np.repeat(sp[:, 0::2], 2, axis=1)     # [T, D]
    sind = np.repeat(sp[:, 1::2], 2, axis=1)
    cosb = np.tile(cosd.T, (1, B))               # [D, NTOK]
    sinb = np.tile(sind.T, (1, B))
    cos2 = np.ascontiguousarray(np.concatenate([cosb, cosb], 0))  # [128, NTOK]
    sin2 = np.ascontiguousarray(np.concatenate([sinb, sinb], 0))

    P = np.zeros((D, D), dtype=np.float32)
    P[: D // 2, D // 2 :] = np.eye(D // 2)
    P[D // 2 :, : D // 2] = -np.eye(D // 2)
    pmat = np.zeros((128, 128), dtype=np.float32)
    pmat[:64, :64] = P
    pmat[64:, 64:] = P

    f = np.arange(128)[None, :]
    p = np.arange(128)[:, None]
    dmask = (f >= p).astype(np.float32)          # S^T diag block: keep tk<=tq

    ident2 = np.concatenate([np.eye(64), np.eye(64)], 0).astype(np.float32)

    shared = {
        "xT": xT, "cos2": cos2, "sin2": sin2,
        "pmat": pmat, "dmask": dmask, "ident2": ident2,
        "vones": np.ones((128, 2 * (T // 128)), dtype=np.float32),
    }
    per_core = []
    for c in range(8):
        # q head h uses kv head h % KVH (jnp.tile), so core c owns
        # q heads {c, c+8, c+16, c+24} and kv head c.
        heads = [c + KVH * g for g in range(HPC)]
        qrows = np.concatenate([np.arange(D * h, D * (h + 1)) for h in heads])
        Wq_c = Wq[qrows]
        Wk_c = Wk[D * c : D * (c + 1)]
        Wv_c = Wv[D * c : D * (c + 1)]
        wcatT = np.ascontiguousarray(
            np.concatenate([Wq_c, Wk_c, Wv_c], 0).T
        ).astype(np.float32)
        bcat = np.concatenate(
            [bq[qrows], bk[D * c : D * (c + 1)], bv[D * c : D * (c + 1)]]
        ).astype(np.float32)
        bqkv = np.ascontiguousarray(bcat.reshape(3, 128).T)
        woR = np.ascontiguousarray(Wo[:, qrows].T).astype(np.float32)
        per_core.append(dict(shared, wcatT=wcatT, bqkv=bqkv, woR=woR))
    return per_core


def kernel(x, mask, sinusoidal_pos, Wq, bq, Wk, bk, Wv, bv, Wo, bo):
    x = np.asarray(x, dtype=np.float32)
    in_maps = _host_inputs(
        x, sinusoidal_pos,
        np.asarray(Wq, np.float32), np.asarray(bq, np.float32),
        np.asarray(Wk, np.float32), np.asarray(bk, np.float32),
        np.asarray(Wv, np.float32), np.asarray(bv, np.float32),
        np.asarray(Wo, np.float32),
    )
    if "nc" not in _NC_CACHE:
        _NC_CACHE["nc"] = build_nc()
    res = run_bass_kernel_spmd(
        _NC_CACHE["nc"], in_maps, core_ids=list(range(8))
    )
    y = np.zeros((NTOK, C), dtype=np.float32)
    for r in res.results:
        y += r["y"]
    y += np.asarray(bo, np.float32)[None, :]
    return y.reshape(B, T, C)


# revision 16
# speedup vs baseline: 1.1215x; 1.1215x over previous
"""Trainium2 Bass kernel for GQA multi-head attention with RoPE.

Sharding: tensor-parallel over heads. Core c owns q-heads 4c..4c+3 and
kv-head c. Each core computes its QKV projection slice, RoPE, causal
attention for its heads, and a partial output projection
(attn_out_local @ Wo[:, local].T). The host sums the 8 partial y's.

Device layouts (per core):
  xT      [C, B*T]   x transposed (replicated)
  wcatT   [C, 384]   [Wq_loc | Wk_loc | Wv_loc].T
  qT/kT   [d, tok]   head-dim on partitions ("T-layout")
  scores  S^T [tk partitions, tq free] so no transposes are needed:
          exp(S^T) feeds P@V directly as the moving operand with
          v_aug = [v | ones] stationary; the ones row yields softmax
          row-sums in partition 64 of the PV accumulator.
  y       [B*T, C]   partial; host adds the 8 partials + bo.

All matmul operands are float32r (fp32 with 11-bit mantissa, full PE
rate at N>=256); PSUM accumulation is fp32.
"""

import sys

sys.path.insert(0, "/opt/trn_rl_repo")

import numpy as np

import bass_rust
import concourse.bass as bass
import concourse.tile as tile
from concourse import mybir
from concourse.bass_utils import run_bass_kernel_spmd

B, T, C = 2, 2048, 2048
H, KVH, D = 32, 8, 64
NTOK = B * T                 # 4096
HPC = H // 8                 # 4 q heads per core
QL = HPC * D                 # 256 local q dims
KC = C // 128                # 16 contraction chunks
SCALE = float(D) ** -0.5

F32 = mybir.dt.float32
F32R = mybir.dt.float32r
AF = mybir.ActivationFunctionType

_NC_CACHE = {}


def _split_waits(nc, limit=1):
    """Walrus in this toolchain allows only one sync-wait per instruction.

    Tile emits instructions with several sem waits (drain/barrier, phase
    boundaries). Hoist the excess onto same-engine NoOps inserted right
    before the instruction — program order on the engine queue preserves
    the wait semantics.
    """
    ctr = 0
    for f in nc.m.functions:
        for blk in f.blocks:
            out = []
            changed = False
            for inst in list(blk.instructions):
                si = inst.sync_info
                if si is not None and len(si.on_wait) > limit:
                    waits = list(si.on_wait)
                    keep, excess = waits[:limit], waits[limit:]
                    for i in range(0, len(excess), limit):
                        ctr += 1
                        nop = mybir.InstNoOp(
                            name=f"I-wsplit-{ctr}", ins=[], outs=[]
                        )
                        nop.engine = inst.engine
                        nop.sync_info = bass_rust.SyncInfo(
                            on_wait=excess[i : i + limit], on_update=[]
                        )
                        out.append(nop)
                        changed = True
                    inst.sync_info = bass_rust.SyncInfo(
                        on_wait=keep, on_update=list(si.on_update)
                    )
                out.append(inst)
            if changed:
                blk.instructions = out
    return ctr


def build_nc():
    nc = bass.Bass(trn_type="TRN2")

    xT_d = nc.dram_tensor("xT", [C, NTOK], F32R, kind="ExternalInput")
    wcat_d = nc.dram_tensor("wcatT", [C, 384], F32R, kind="ExternalInput")
    bias_d = nc.dram_tensor("bqkv", [128, 3], F32, kind="ExternalInput")
    wo_d = nc.dram_tensor("woR", [QL, C], F32R, kind="ExternalInput")
    cos_d = nc.dram_tensor("cos2", [128, NTOK], F32R, kind="ExternalInput")
    sin_d = nc.dram_tensor("sin2", [128, NTOK], F32R, kind="ExternalInput")
    pmat_d = nc.dram_tensor("pmat", [128, 128], F32R, kind="ExternalInput")
    dmask_d = nc.dram_tensor("dmask", [128, 128], F32R, kind="ExternalInput")
    ident_d = nc.dram_tensor("ident2", [128, 64], F32R, kind="ExternalInput")
    vones_d = nc.dram_tensor(
        "vones", [128, 2 * (T // 128)], F32R, kind="ExternalInput"
    )
    y_d = nc.dram_tensor("y", [NTOK, C], F32, kind="ExternalOutput")

    with tile.TileContext(nc) as tc:
        with (
            tc.tile_pool(name="consts", bufs=1) as consts,
            tc.tile_pool(name="xs", bufs=2) as xs_pool,
            tc.tile_pool(name="acts", bufs=1) as acts,
            tc.tile_pool(name="big", bufs=2) as big,
            tc.tile_pool(name="tmp", bufs=2) as tmp_pool,
            tc.tile_pool(name="es", bufs=2) as es_pool,
            tc.tile_pool(name="rows", bufs=2) as rows,
            tc.tile_pool(name="ibc", bufs=2) as ibc_pool,
            tc.tile_pool(name="onorm", bufs=2) as on_pool,
            tc.tile_pool(name="ystage", bufs=4) as y_pool,
            tc.tile_pool(name="psA", bufs=2, space="PSUM") as psA,
            tc.tile_pool(name="psB", bufs=2, space="PSUM") as psB,
        ):
            # ---- constants ----
            wcat_sb = consts.tile([128, KC, 384], F32R, tag="wcat")
            for k in range(KC):
                nc.sync.dma_start(
                    out=wcat_sb[:, k, :], in_=wcat_d[128 * k : 128 * (k + 1), :]
                )
            bias_sb = consts.tile([128, 3], F32, tag="bias")
            nc.sync.dma_start(out=bias_sb, in_=bias_d[:, :])
            wo_sb = consts.tile([128, 2, C], F32R, tag="wo")
            for k in range(2):
                nc.sync.dma_start(
                    out=wo_sb[:, k, :], in_=wo_d[128 * k : 128 * (k + 1), :]
                )
            pmat_sb = consts.tile([128, 128], F32R, tag="pmat")
            nc.sync.dma_start(out=pmat_sb, in_=pmat_d[:, :])
            dmask_sb = consts.tile([128, 128], F32R, tag="dmask")
            nc.sync.dma_start(out=dmask_sb, in_=dmask_d[:, :])
            ident_sb = consts.tile([128, 64], F32R, tag="ident")
            nc.sync.dma_start(out=ident_sb, in_=ident_d[:, :])

            cos_sb = big.tile([128, NTOK], F32R, tag="big")
            nc.sync.dma_start(out=cos_sb, in_=cos_d[:, :])
            sin_sb = big.tile([128, NTOK], F32R, tag="big")
            nc.sync.dma_start(out=sin_sb, in_=sin_d[:, :])

            qT01 = acts.tile([128, NTOK], F32R, tag="qT01")
            qT23 = acts.tile([128, NTOK], F32R, tag="qT23")
            kvT = acts.tile([128, NTOK], F32R, tag="kvT")
            ktdup = acts.tile([128, NTOK], F32R, tag="ktdup")
            vaug = acts.tile([128, 2 * (T // 128), 65], F32R, tag="vaug")

            # ---- phase 1: QKV projection + RoPE + v-transpose, fused ----
            # Processing 1024-token groups keeps the PE stream dense: the
            # RoPE rotate-matmuls and v-transposes of group g interleave
            # with the QKV matmuls of group g+1, so the HAM never
            # re-throttles between phases.
            nc.sync.dma_start(
                out=vaug[:, :, 64:65], in_=vones_d[:, :].unsqueeze(2)
            )
            qkv_dst = [qT01, qT23, kvT]
            NJ = T // 128  # 16
            for ng in range(NTOK // 1024):
                base = 1024 * ng
                ps0 = psA.tile([128, 1024], F32, tag="a")
                ps1 = psA.tile([128, 1024], F32, tag="a")
                ps2 = psB.tile([128, 1024], F32, tag="o")
                pss = [ps0, ps1, ps2]
                for k in range(KC):
                    xt = xs_pool.tile([128, 1024], F32R, tag="xs")
                    nc.sync.dma_start(
                        out=xt,
                        in_=xT_d[128 * k : 128 * (k + 1), base : base + 1024],
                    )
                    for m in range(3):
                        for c2 in range(2):
                            nc.tensor.matmul(
                                pss[m][:, 512 * c2 : 512 * (c2 + 1)],
                                wcat_sb[:, k, 128 * m : 128 * (m + 1)],
                                xt[:, 512 * c2 : 512 * (c2 + 1)],
                                start=(k == 0),
                                stop=(k == KC - 1),
                            )
                for m in range(3):
                    nc.scalar.activation(
                        out=qkv_dst[m][:, base : base + 1024],
                        in_=pss[m],
                        func=AF.Identity,
                        bias=bias_sb[:, m : m + 1],
                        scale=1.0,
                    )
                # RoPE for this token group (token-pointwise)
                for dst, rn in ((qT01, 128), (qT23, 128), (kvT, 64)):
                    rot = psB.tile([128, 1024], F32, tag="o")
                    for c2 in range(2):
                        nc.tensor.matmul(
                            rot[:rn, 512 * c2 : 512 * (c2 + 1)],
                            pmat_sb[:rn, :rn],
                            dst[:rn, base + 512 * c2 : base + 512 * (c2 + 1)],
                            start=True,
                            stop=True,
                        )
                    tmp = tmp_pool.tile([128, 1024], F32, tag="tmp")
                    nc.vector.tensor_mul(
                        tmp[:rn], rot[:rn, :], sin_sb[:rn, base : base + 1024]
                    )
                    nc.vector.tensor_mul(
                        dst[:rn, base : base + 1024],
                        dst[:rn, base : base + 1024],
                        cos_sb[:rn, base : base + 1024],
                    )
                    nc.vector.tensor_add(
                        dst[:rn, base : base + 1024],
                        dst[:rn, base : base + 1024],
                        tmp[:rn],
                    )
                # v transposes for this token group (v is not roped)
                b2 = ng // 2
                for jj in range(8):
                    jt = (ng % 2) * 8 + jj
                    vps = psB.tile([128, 64], F32R, tag="o")
                    nc.tensor.transpose(
                        vps,
                        kvT[64:128, T * b2 + 128 * jt : T * b2 + 128 * (jt + 1)],
                        ident_sb[64:128, :],
                    )
                    nc.vector.tensor_copy(
                        vaug[:, b2 * NJ + jt, 0:64], vps
                    )

            # duplicate roped kT into partitions 64:128 for odd heads
            nc.sync.dma_start(out=ktdup[64:128, :], in_=kvT[0:64, :])

            # ---- phase 3: attention, per (batch, head, tq-half) ----
            OT0 = big.tile([128, NTOK], F32R, tag="big")
            OT1 = big.tile([128, NTOK], F32R, tag="big")
            OT = [OT0, OT1]
            for b2 in range(B):
                for hq in range(HPC):
                    qtile = qT01 if hq < 2 else qT23
                    qr = 64 * (hq % 2)
                    kt = kvT if hq % 2 == 0 else ktdup
                    kr = 0 if hq % 2 == 0 else 64
                    for v2 in range(2):
                        tq0 = 1024 * v2
                        jmax = 8 * v2 + 8
                        ops = psB.tile([65, 1024], F32, tag="o")
                        last_bank = [8 * v2 + 3, jmax - 1]
                        for j in range(jmax):
                            tqs = max(128 * j, tq0)
                            W = tq0 + 1024 - tqs
                            sps = psA.tile([128, 1024], F32, tag="a")
                            off = 0
                            while off < W:
                                nw = min(512, W - off)
                                nc.tensor.matmul(
                                    sps[:, off : off + nw],
                                    kt[kr : kr + 64,
                                       T * b2 + 128 * j : T * b2 + 128 * (j + 1)],
                                    qtile[qr : qr + 64,
                                          T * b2 + tqs + off :
                                          T * b2 + tqs + off + nw],
                                    start=True,
                                    stop=True,
                                )
                                off += nw
                            es = es_pool.tile([128, 1024], F32R, tag="es")
                            nc.scalar.activation(
                                out=es[:, :W], in_=sps[:, :W],
                                func=AF.Exp, scale=SCALE,
                            )
                            if 128 * j >= tq0:
                                nc.vector.tensor_mul(
                                    es[:, 0:128], es[:, 0:128], dmask_sb
                                )
                            off = 0
                            while off < W:
                                oc = tqs - tq0 + off
                                bank = oc // 512
                                nw = min(512 - oc % 512, W - off)
                                nc.tensor.matmul(
                                    ops[:, oc : oc + nw],
                                    vaug[:, b2 * NJ + j, :],
                                    es[:, off : off + nw],
                                    start=(j == 0),
                                    stop=(j == last_bank[bank]),
                                )
                                off += nw
                        # softmax denominators: 1/rowsum via exp(-ln(.))
                        lnr = rows.tile([1, 1024], F32, tag="lnr")
                        nc.scalar.activation(
                            out=lnr, in_=ops[64:65, :], func=AF.Ln
                        )
                        nc.scalar.activation(
                            out=lnr, in_=lnr, func=AF.Exp, scale=-1.0
                        )
                        scr = nc.dram_tensor(
                            f"scr_{b2}_{hq}_{v2}", [1, 1024], F32
                        )
                        nc.sync.dma_start(out=scr[:, :], in_=lnr)
                        ibc = ibc_pool.tile([64, 1024], F32, tag="ibc")
                        nc.sync.dma_start(
                            out=ibc,
                            in_=scr[:, :].partition_broadcast(64).squeeze(1),
                        )
                        ot = OT[hq // 2]
                        tok0 = T * b2 + tq0
                        if hq % 2 == 0:
                            nc.vector.tensor_mul(
                                ot[0:64, tok0 : tok0 + 1024],
                                ops[0:64, :],
                                ibc,
                            )
                        else:
                            on = on_pool.tile([64, 1024], F32R, tag="on")
                            nc.vector.tensor_mul(on, ops[0:64, :], ibc)
                            nc.sync.dma_start(
                                out=ot[64:128, tok0 : tok0 + 1024], in_=on
                            )

            # ---- phase 4: partial output projection ----
            for mt in range(NTOK // 128):
                for ncc in range(C // 512):
                    pool = psA if ncc % 2 == 0 else psB
                    tag = "a" if ncc % 2 == 0 else "o"
                    yp = pool.tile([128, 512], F32, tag=tag)
                    for kq in range(2):
                        nc.tensor.matmul(
                            yp,
                            OT[kq][:, 128 * mt : 128 * (mt + 1)],
                            wo_sb[:, kq, 512 * ncc : 512 * (ncc + 1)],
                            start=(kq == 0),
                            stop=(kq == 1),
                        )
                    ys = y_pool.tile([128, 512], F32, tag="ys")
                    if ncc % 2 == 0:
                        nc.scalar.copy(ys, yp)
                    else:
                        nc.vector.tensor_copy(ys, yp)
                    nc.sync.dma_start(
                        out=y_d[128 * mt : 128 * (mt + 1),
                                512 * ncc : 512 * (ncc + 1)],
                        in_=ys,
                    )
    _split_waits(nc)
    return nc


def _host_inputs(x, sinusoidal_pos, Wq, bq, Wk, bk, Wv, bv, Wo):
    xT = np.ascontiguousarray(x.reshape(NTOK, C).T).astype(np.float32)

    sp = np.asarray(sinusoidal_pos, dtype=np.float32).reshape(T, D)
    cosd = np.repeat(sp[:, 0::2], 2, axis=1)     # [T, D]
    sind = np.repeat(sp[:, 1::2], 2, axis=1)
    cosb = np.tile(cosd.T, (1, B))               # [D, NTOK]
    sinb = np.tile(sind.T, (1, B))
    cos2 = np.ascontiguousarray(np.concatenate([cosb, cosb], 0))  # [128, NTOK]
    sin2 = np.ascontiguousarray(np.concatenate([sinb, sinb], 0))

    P = np.zeros((D, D), dtype=np.float32)
    P[: D // 2, D // 2 :] = np.eye(D // 2)
    P[D // 2 :, : D // 2] = -np.eye(D // 2)
    pmat = np.zeros((128, 128), dtype=np.float32)
    pmat[:64, :64] = P
    pmat[64:, 64:] = P

    f = np.arange(128)[None, :]
    p = np.arange(128)[:, None]
    dmask = (f >= p).astype(np.float32)          # S^T diag block: keep tk<=tq

    ident2 = np.concatenate([np.eye(64), np.eye(64)], 0).astype(np.float32)

    shared = {
        "xT": xT, "cos2": cos2, "sin2": sin2,
        "pmat": pmat, "dmask": dmask, "ident2": ident2,
        "vones": np.ones((128, 2 * (T // 128)), dtype=np.float32),
    }
    per_core = []
    for c in range(8):
        # q head h uses kv head h % KVH (jnp.tile), so core c owns
        # q heads {c, c+8, c+16, c+24} and kv head c.
        heads = [c + KVH * g for g in range(HPC)]
        qrows = np.concatenate([np.arange(D * h, D * (h + 1)) for h in heads])
        Wq_c = Wq[qrows]
        Wk_c = Wk[D * c : D * (c + 1)]
        Wv_c = Wv[D * c : D * (c + 1)]
        wcatT = np.ascontiguousarray(
            np.concatenate([Wq_c, Wk_c, Wv_c], 0).T
        ).astype(np.float32)
        bcat = np.concatenate(
            [bq[qrows], bk[D * c : D * (c + 1)], bv[D * c : D * (c + 1)]]
        ).astype(np.float32)
        bqkv = np.ascontiguousarray(bcat.reshape(3, 128).T)
        woR = np.ascontiguousarray(Wo[:, qrows].T).astype(np.float32)
        per_core.append(dict(shared, wcatT=wcatT, bqkv=bqkv, woR=woR))
    return per_core


def kernel(x, mask, sinusoidal_pos, Wq, bq, Wk, bk, Wv, bv, Wo, bo):
    x = np.asarray(x, dtype=np.float32)
    in_maps = _host_inputs(
        x, sinusoidal_pos,
        np.asarray(Wq, np.float32), np.asarray(bq, np.float32),
        np.asarray(Wk, np.float32), np.asarray(bk, np.float32),
        np.asarray(Wv, np.float32), np.asarray(bv, np.float32),
        np.asarray(Wo, np.float32),
    )
    if "nc" not in _NC_CACHE:
        _NC_CACHE["nc"] = build_nc()
    res = run_bass_kernel_spmd(
        _NC_CACHE["nc"], in_maps, core_ids=list(range(8))
    )
    y = np.zeros((NTOK, C), dtype=np.float32)
    for r in res.results:
        y += r["y"]
    y += np.asarray(bo, np.float32)[None, :]
    return y.reshape(B, T, C)


# revision 17
# speedup vs baseline: 1.1622x; 1.0362x over previous
"""Trainium2 Bass kernel for GQA multi-head attention with RoPE.

Sharding: tensor-parallel over heads. Core c owns q-heads 4c..4c+3 and
kv-head c. Each core computes its QKV projection slice, RoPE, causal
attention for its heads, and a partial output projection
(attn_out_local @ Wo[:, local].T). The host sums the 8 partial y's.

Device layouts (per core):
  xT      [C, B*T]   x transposed (replicated)
  wcatT   [C, 384]   [Wq_loc | Wk_loc | Wv_loc].T
  qT/kT   [d, tok]   head-dim on partitions ("T-layout")
  scores  S^T [tk partitions, tq free] so no transposes are needed:
          exp(S^T) feeds P@V directly as the moving operand with
          v_aug = [v | ones] stationary; the ones row yields softmax
          row-sums in partition 64 of the PV accumulator.
  y       [B*T, C]   partial; host adds the 8 partials + bo.

All matmul operands are float32r (fp32 with 11-bit mantissa, full PE
rate at N>=256); PSUM accumulation is fp32.
"""

import sys

sys.path.insert(0, "/opt/trn_rl_repo")

import numpy as np

import bass_rust
import concourse.bass as bass
import concourse.tile as tile
from concourse import mybir
from concourse.bass_utils import run_bass_kernel_spmd

B, T, C = 2, 2048, 2048
H, KVH, D = 32, 8, 64
NTOK = B * T                 # 4096
HPC = H // 8                 # 4 q heads per core
QL = HPC * D                 # 256 local q dims
KC = C // 128                # 16 contraction chunks
SCALE = float(D) ** -0.5

F32 = mybir.dt.float32
F32R = mybir.dt.float32r
AF = mybir.ActivationFunctionType

_NC_CACHE = {}


def _split_waits(nc, limit=1):
    """Walrus in this toolchain allows only one sync-wait per instruction.

    Tile emits instructions with several sem waits (drain/barrier, phase
    boundaries). Hoist the excess onto same-engine NoOps inserted right
    before the instruction — program order on the engine queue preserves
    the wait semantics.
    """
    ctr = 0
    for f in nc.m.functions:
        for blk in f.blocks:
            out = []
            changed = False
            for inst in list(blk.instructions):
                si = inst.sync_info
                if si is not None and len(si.on_wait) > limit:
                    waits = list(si.on_wait)
                    keep, excess = waits[:limit], waits[limit:]
                    for i in range(0, len(excess), limit):
                        ctr += 1
                        nop = mybir.InstNoOp(
                            name=f"I-wsplit-{ctr}", ins=[], outs=[]
                        )
                        nop.engine = inst.engine
                        nop.sync_info = bass_rust.SyncInfo(
                            on_wait=excess[i : i + limit], on_update=[]
                        )
                        out.append(nop)
                        changed = True
                    inst.sync_info = bass_rust.SyncInfo(
                        on_wait=keep, on_update=list(si.on_update)
                    )
                out.append(inst)
            if changed:
                blk.instructions = out
    return ctr


def build_nc():
    nc = bass.Bass(trn_type="TRN2")

    xT_d = nc.dram_tensor("xT", [C, NTOK], F32R, kind="ExternalInput")
    wcat_d = nc.dram_tensor("wcatT", [C, 384], F32R, kind="ExternalInput")
    bias_d = nc.dram_tensor("bqkv", [128, 3], F32, kind="ExternalInput")
    wo_d = nc.dram_tensor("woR", [QL, C], F32R, kind="ExternalInput")
    cos_d = nc.dram_tensor("cos2", [128, NTOK], F32R, kind="ExternalInput")
    sin_d = nc.dram_tensor("sin2", [128, NTOK], F32R, kind="ExternalInput")
    pmat_d = nc.dram_tensor("pmat", [128, 128], F32R, kind="ExternalInput")
    dmask_d = nc.dram_tensor("dmask", [128, 128], F32R, kind="ExternalInput")
    ident_d = nc.dram_tensor("ident2", [128, 64], F32R, kind="ExternalInput")
    vones_d = nc.dram_tensor(
        "vones", [128, 2 * (T // 128)], F32R, kind="ExternalInput"
    )
    y_d = nc.dram_tensor("y", [NTOK, C], F32, kind="ExternalOutput")

    with tile.TileContext(nc) as tc:
        with (
            tc.tile_pool(name="consts", bufs=1) as consts,
            tc.tile_pool(name="xs", bufs=2) as xs_pool,
            tc.tile_pool(name="acts", bufs=1) as acts,
            tc.tile_pool(name="big", bufs=2) as big,
            tc.tile_pool(name="tmp", bufs=2) as tmp_pool,
            tc.tile_pool(name="es", bufs=2) as es_pool,
            tc.tile_pool(name="rows", bufs=2) as rows,
            tc.tile_pool(name="ibc", bufs=2) as ibc_pool,
            tc.tile_pool(name="onorm", bufs=1) as on_pool,
            tc.tile_pool(name="ystage", bufs=4) as y_pool,
            tc.tile_pool(name="psA", bufs=2, space="PSUM") as psA,
            tc.tile_pool(name="psB", bufs=2, space="PSUM") as psB,
        ):
            # ---- constants ----
            wcat_sb = consts.tile([128, KC, 384], F32R, tag="wcat")
            for k in range(KC):
                nc.sync.dma_start(
                    out=wcat_sb[:, k, :], in_=wcat_d[128 * k : 128 * (k + 1), :]
                )
            bias_sb = consts.tile([128, 3], F32, tag="bias")
            nc.sync.dma_start(out=bias_sb, in_=bias_d[:, :])
            wo_sb = consts.tile([128, 2, C], F32R, tag="wo")
            for k in range(2):
                nc.sync.dma_start(
                    out=wo_sb[:, k, :], in_=wo_d[128 * k : 128 * (k + 1), :]
                )
            pmat_sb = consts.tile([128, 128], F32R, tag="pmat")
            nc.sync.dma_start(out=pmat_sb, in_=pmat_d[:, :])
            dmask_sb = consts.tile([128, 128], F32R, tag="dmask")
            nc.sync.dma_start(out=dmask_sb, in_=dmask_d[:, :])
            ident_sb = consts.tile([128, 64], F32R, tag="ident")
            nc.sync.dma_start(out=ident_sb, in_=ident_d[:, :])

            cos_sb = big.tile([128, NTOK], F32R, tag="big")
            nc.sync.dma_start(out=cos_sb, in_=cos_d[:, :])
            sin_sb = big.tile([128, NTOK], F32R, tag="big")
            nc.sync.dma_start(out=sin_sb, in_=sin_d[:, :])

            qT01 = acts.tile([128, NTOK], F32R, tag="qT01")
            qT23 = acts.tile([128, NTOK], F32R, tag="qT23")
            kvT = acts.tile([128, NTOK], F32R, tag="kvT")
            ktdup = acts.tile([128, NTOK], F32R, tag="ktdup")
            vaug = acts.tile([128, 2 * (T // 128), 65], F32R, tag="vaug")

            # ---- phase 1: QKV projection + RoPE + v-transpose, fused ----
            # Processing 1024-token groups keeps the PE stream dense: the
            # RoPE rotate-matmuls and v-transposes of group g interleave
            # with the QKV matmuls of group g+1, so the HAM never
            # re-throttles between phases.
            nc.sync.dma_start(
                out=vaug[:, :, 64:65], in_=vones_d[:, :].unsqueeze(2)
            )
            qkv_dst = [qT01, qT23, kvT]
            NJ = T // 128  # 16
            for ng in range(NTOK // 1024):
                base = 1024 * ng
                ps0 = psA.tile([128, 1024], F32, tag="a")
                ps1 = psA.tile([128, 1024], F32, tag="a")
                ps2 = psB.tile([128, 1024], F32, tag="o")
                pss = [ps0, ps1, ps2]
                for k in range(KC):
                    xt = xs_pool.tile([128, 1024], F32R, tag="xs")
                    nc.sync.dma_start(
                        out=xt,
                        in_=xT_d[128 * k : 128 * (k + 1), base : base + 1024],
                    )
                    for m in range(3):
                        for c2 in range(2):
                            nc.tensor.matmul(
                                pss[m][:, 512 * c2 : 512 * (c2 + 1)],
                                wcat_sb[:, k, 128 * m : 128 * (m + 1)],
                                xt[:, 512 * c2 : 512 * (c2 + 1)],
                                start=(k == 0),
                                stop=(k == KC - 1),
                            )
                for m in range(3):
                    nc.scalar.activation(
                        out=qkv_dst[m][:, base : base + 1024],
                        in_=pss[m],
                        func=AF.Identity,
                        bias=bias_sb[:, m : m + 1],
                        scale=1.0,
                    )
                # RoPE for this token group (token-pointwise)
                for dst, rn in ((qT01, 128), (qT23, 128), (kvT, 64)):
                    rot = psB.tile([128, 1024], F32, tag="o")
                    for c2 in range(2):
                        nc.tensor.matmul(
                            rot[:rn, 512 * c2 : 512 * (c2 + 1)],
                            pmat_sb[:rn, :rn],
                            dst[:rn, base + 512 * c2 : base + 512 * (c2 + 1)],
                            start=True,
                            stop=True,
                        )
                    tmp = tmp_pool.tile([128, 1024], F32, tag="tmp")
                    nc.vector.tensor_mul(
                        tmp[:rn], rot[:rn, :], sin_sb[:rn, base : base + 1024]
                    )
                    nc.vector.tensor_mul(
                        dst[:rn, base : base + 1024],
                        dst[:rn, base : base + 1024],
                        cos_sb[:rn, base : base + 1024],
                    )
                    nc.vector.tensor_add(
                        dst[:rn, base : base + 1024],
                        dst[:rn, base : base + 1024],
                        tmp[:rn],
                    )
                # v transposes for this token group (v is not roped)
                b2 = ng // 2
                for jj in range(8):
                    jt = (ng % 2) * 8 + jj
                    vps = psB.tile([128, 64], F32R, tag="o")
                    nc.tensor.transpose(
                        vps,
                        kvT[64:128, T * b2 + 128 * jt : T * b2 + 128 * (jt + 1)],
                        ident_sb[64:128, :],
                    )
                    nc.vector.tensor_copy(
                        vaug[:, b2 * NJ + jt, 0:64], vps
                    )

            # duplicate roped kT into partitions 64:128 for odd heads
            nc.sync.dma_start(out=ktdup[64:128, :], in_=kvT[0:64, :])

            # ---- phase 3: attention, per (batch, head, tq-half) ----
            OT0 = big.tile([128, NTOK], F32R, tag="big")
            OT1 = big.tile([128, NTOK], F32R, tag="big")
            OT = [OT0, OT1]
            for b2 in range(B):
                for hp in range(2):  # head pair (2hp, 2hp+1)
                    qtile = OTq = [qT01, qT23][hp]
                    for v2 in range(2):
                        tq0 = 1024 * v2
                        jmax = 8 * v2 + 8
                        ops_e = psB.tile([65, 1024], F32, tag="o")
                        ops_o = psB.tile([65, 1024], F32, tag="o")
                        last_bank = [8 * v2 + 3, jmax - 1]
                        for j in range(jmax):
                            tqs = max(128 * j, tq0)
                            W = tq0 + 1024 - tqs
                            ksl = slice(T * b2 + 128 * j, T * b2 + 128 * (j + 1))
                            sps_e = psA.tile([128, 1024], F32, tag="a")
                            sps_o = psA.tile([128, 1024], F32, tag="a")
                            off = 0
                            while off < W:
                                nw = min(512, W - off)
                                qsl = slice(T * b2 + tqs + off,
                                            T * b2 + tqs + off + nw)
                                # even head rows 0:64, odd head rows 64:128 —
                                # adjacent issue -> concurrent PE row groups
                                nc.tensor.matmul(
                                    sps_e[:, off : off + nw],
                                    kvT[0:64, ksl], qtile[0:64, qsl],
                                    start=True, stop=True,
                                )
                                nc.tensor.matmul(
                                    sps_o[:, off : off + nw],
                                    ktdup[64:128, ksl], qtile[64:128, qsl],
                                    start=True, stop=True,
                                )
                                off += nw
                            es_e = es_pool.tile([128, 1024], F32R, tag="ese")
                            es_o = es_pool.tile([128, 1024], F32R, tag="eso")
                            nc.scalar.activation(
                                out=es_e[:, :W], in_=sps_e[:, :W],
                                func=AF.Exp, scale=SCALE,
                            )
                            nc.scalar.activation(
                                out=es_o[:, :W], in_=sps_o[:, :W],
                                func=AF.Exp, scale=SCALE,
                            )
                            if 128 * j >= tq0:
                                nc.vector.tensor_mul(
                                    es_e[:, 0:128], es_e[:, 0:128], dmask_sb
                                )
                                nc.vector.tensor_mul(
                                    es_o[:, 0:128], es_o[:, 0:128], dmask_sb
                                )
                            off = 0
                            while off < W:
                                oc = tqs - tq0 + off
                                bank = oc // 512
                                nw = min(512 - oc % 512, W - off)
                                for ops, es in ((ops_e, es_e), (ops_o, es_o)):
                                    nc.tensor.matmul(
                                        ops[:, oc : oc + nw],
                                        vaug[:, b2 * NJ + j, :],
                                        es[:, off : off + nw],
                                        start=(j == 0),
                                        stop=(j == last_bank[bank]),
                                    )
                                off += nw
                        # softmax denominators: 1/rowsum via exp(-ln(.))
                        tok0 = T * b2 + tq0
                        for par, ops in ((0, ops_e), (1, ops_o)):
                            lnr = rows.tile([1, 1024], F32, tag="lnr")
                            nc.scalar.activation(
                                out=lnr, in_=ops[64:65, :], func=AF.Ln
                            )
                            nc.scalar.activation(
                                out=lnr, in_=lnr, func=AF.Exp, scale=-1.0
                            )
                            scr = nc.dram_tensor(
                                f"scr_{b2}_{hp}_{par}_{v2}", [1, 1024], F32
                            )
                            nc.sync.dma_start(out=scr[:, :], in_=lnr)
                            ibc = ibc_pool.tile([64, 1024], F32, tag="ibc")
                            nc.sync.dma_start(
                                out=ibc,
                                in_=scr[:, :].partition_broadcast(64).squeeze(1),
                            )
                            ot = OT[hp]
                            if par == 0:
                                nc.vector.tensor_mul(
                                    ot[0:64, tok0 : tok0 + 1024],
                                    ops[0:64, :],
                                    ibc,
                                )
                            else:
                                on = on_pool.tile([64, 1024], F32R, tag="on")
                                nc.vector.tensor_mul(on, ops[0:64, :], ibc)
                                nc.sync.dma_start(
                                    out=ot[64:128, tok0 : tok0 + 1024], in_=on
                                )

            # ---- phase 4: partial output projection ----
            for mt in range(NTOK // 128):
                for ncc in range(C // 512):
                    pool = psA if ncc % 2 == 0 else psB
                    tag = "a" if ncc % 2 == 0 else "o"
                    yp = pool.tile([128, 512], F32, tag=tag)
                    for kq in range(2):
                        nc.tensor.matmul(
                            yp,
                            OT[kq][:, 128 * mt : 128 * (mt + 1)],
                            wo_sb[:, kq, 512 * ncc : 512 * (ncc + 1)],
                            start=(kq == 0),
                            stop=(kq == 1),
                        )
                    ys = y_pool.tile([128, 512], F32, tag="ys")
                    if ncc % 2 == 0:
                        nc.scalar.copy(ys, yp)
                    else:
                        nc.vector.tensor_copy(ys, yp)
                    nc.sync.dma_start(
                        out=y_d[128 * mt : 128 * (mt + 1),
                                512 * ncc : 512 * (ncc + 1)],
                        in_=ys,
                    )
    _split_waits(nc)
    return nc


def _host_inputs(x, sinusoidal_pos, Wq, bq, Wk, bk, Wv, bv, Wo):
    xT = np.ascontiguousarray(x.reshape(NTOK, C).T).astype(np.float32)

    sp = np.asarray(sinusoidal_pos, dtype=np.float32).reshape(T, D)
    cosd = np.repeat(sp[:, 0::2], 2, axis=1)     # [T, D]
    sind = np.repeat(sp[:, 1::2], 2, axis=1)
    cosb = np.tile(cosd.T, (1, B))               # [D, NTOK]
    sinb = np.tile(sind.T, (1, B))
    cos2 = np.ascontiguousarray(np.concatenate([cosb, cosb], 0))  # [128, NTOK]
    sin2 = np.ascontiguousarray(np.concatenate([sinb, sinb], 0))

    P = np.zeros((D, D), dtype=np.float32)
    P[: D // 2, D // 2 :] = np.eye(D // 2)
    P[D // 2 :, : D // 2] = -np.eye(D // 2)
    pmat = np.zeros((128, 128), dtype=np.float32)
    pmat[:64, :64] = P
    pmat[64:, 64:] = P

    f = np.arange(128)[None, :]
    p = np.arange(128)[:, None]
    dmask = (f >= p).astype(np.float32)          # S^T diag block: keep tk<=tq

    ident2 = np.concatenate([np.eye(64), np.eye(64)], 0).astype(np.float32)

    shared = {
        "xT": xT, "cos2": cos2, "sin2": sin2,
        "pmat": pmat, "dmask": dmask, "ident2": ident2,
        "vones": np.ones((128, 2 * (T // 128)), dtype=np.float32),
    }
    per_core = []
    for c in range(8):
        # q head h uses kv head h % KVH (jnp.tile), so core c owns
        # q heads {c, c+8, c+16, c+24} and kv head c.
        heads = [c + KVH * g for g in range(HPC)]
        qrows = np.concatenate([np.arange(D * h, D * (h + 1)) for h in heads])
        Wq_c = Wq[qrows]
        Wk_c = Wk[D * c : D * (c + 1)]
        Wv_c = Wv[D * c : D * (c + 1)]
        wcatT = np.ascontiguousarray(
            np.concatenate([Wq_c, Wk_c, Wv_c], 0).T
        ).astype(np.float32)
        bcat = np.concatenate(
            [bq[qrows], bk[D * c : D * (c + 1)], bv[D * c : D * (c + 1)]]
        ).astype(np.float32)
        bqkv = np.ascontiguousarray(bcat.reshape(3, 128).T)
        woR = np.ascontiguousarray(Wo[:, qrows].T).astype(np.float32)
        per_core.append(dict(shared, wcatT=wcatT, bqkv=bqkv, woR=woR))
    return per_core


def kernel(x, mask, sinusoidal_pos, Wq, bq, Wk, bk, Wv, bv, Wo, bo):
    x = np.asarray(x, dtype=np.float32)
    in_maps = _host_inputs(
        x, sinusoidal_pos,
        np.asarray(Wq, np.float32), np.asarray(bq, np.float32),
        np.asarray(Wk, np.float32), np.asarray(bk, np.float32),
        np.asarray(Wv, np.float32), np.asarray(bv, np.float32),
        np.asarray(Wo, np.float32),
    )
    if "nc" not in _NC_CACHE:
        _NC_CACHE["nc"] = build_nc()
    res = run_bass_kernel_spmd(
        _NC_CACHE["nc"], in_maps, core_ids=list(range(8))
    )
    y = np.zeros((NTOK, C), dtype=np.float32)
    for r in res.results:
        y += r["y"]
    y += np.asarray(bo, np.float32)[None, :]
    return y.reshape(B, T, C)


# revision 19
# speedup vs baseline: 1.4493x; 1.2471x over previous
"""Trainium2 Bass kernel for GQA multi-head attention with RoPE.

Sharding: tensor-parallel over heads. Core c owns q-heads 4c..4c+3 and
kv-head c. Each core computes its QKV projection slice, RoPE, causal
attention for its heads, and a partial output projection
(attn_out_local @ Wo[:, local].T). The host sums the 8 partial y's.

Device layouts (per core):
  xT      [C, B*T]   x transposed (replicated)
  wcatT   [C, 384]   [Wq_loc | Wk_loc | Wv_loc].T
  qT/kT   [d, tok]   head-dim on partitions ("T-layout")
  scores  S^T [tk partitions, tq free] so no transposes are needed:
          exp(S^T) feeds P@V directly as the moving operand with
          v_aug = [v | ones] stationary; the ones row yields softmax
          row-sums in partition 64 of the PV accumulator.
  y       [B*T, C]   partial; host adds the 8 partials + bo.

All matmul operands are float32r (fp32 with 11-bit mantissa, full PE
rate at N>=256); PSUM accumulation is fp32.
"""

import sys

sys.path.insert(0, "/opt/trn_rl_repo")

import numpy as np

import bass_rust
import concourse.bass as bass
import concourse.tile as tile
from concourse import mybir
from concourse.bass_utils import run_bass_kernel_spmd

B, T, C = 2, 2048, 2048
H, KVH, D = 32, 8, 64
NTOK = B * T                 # 4096
HPC = H // 8                 # 4 q heads per core
QL = HPC * D                 # 256 local q dims
KC = C // 128                # 16 contraction chunks
SCALE = float(D) ** -0.5

F32 = mybir.dt.float32
F32R = mybir.dt.float32r
AF = mybir.ActivationFunctionType

_NC_CACHE = {}


def _split_waits(nc, limit=1):
    """Walrus in this toolchain allows only one sync-wait per instruction.

    Tile emits instructions with several sem waits (drain/barrier, phase
    boundaries). Hoist the excess onto same-engine NoOps inserted right
    before the instruction — program order on the engine queue preserves
    the wait semantics.
    """
    ctr = 0
    for f in nc.m.functions:
        for blk in f.blocks:
            out = []
            changed = False
            for inst in list(blk.instructions):
                si = inst.sync_info
                if si is not None and len(si.on_wait) > limit:
                    waits = list(si.on_wait)
                    keep, excess = waits[:limit], waits[limit:]
                    for i in range(0, len(excess), limit):
                        ctr += 1
                        nop = mybir.InstNoOp(
                            name=f"I-wsplit-{ctr}", ins=[], outs=[]
                        )
                        nop.engine = inst.engine
                        nop.sync_info = bass_rust.SyncInfo(
                            on_wait=excess[i : i + limit], on_update=[]
                        )
                        out.append(nop)
                        changed = True
                    inst.sync_info = bass_rust.SyncInfo(
                        on_wait=keep, on_update=list(si.on_update)
                    )
                out.append(inst)
            if changed:
                blk.instructions = out
    return ctr


def build_nc():
    nc = bass.Bass(trn_type="TRN2")

    xT_d = nc.dram_tensor("xT", [C, NTOK], F32R, kind="ExternalInput")
    wcat_d = nc.dram_tensor("wcatT", [C, 384], F32R, kind="ExternalInput")
    bias_d = nc.dram_tensor("bqkv", [128, 3], F32, kind="ExternalInput")
    wo_d = nc.dram_tensor("woR", [QL, C], F32R, kind="ExternalInput")
    cos_d = nc.dram_tensor("cos2", [128, NTOK], F32R, kind="ExternalInput")
    sin_d = nc.dram_tensor("sin2", [128, NTOK], F32R, kind="ExternalInput")
    pmat_d = nc.dram_tensor("pmat", [128, 128], F32R, kind="ExternalInput")
    dmask_d = nc.dram_tensor("dmask", [128, 128], F32R, kind="ExternalInput")
    ident_d = nc.dram_tensor("ident2", [128, 64], F32R, kind="ExternalInput")
    vones_d = nc.dram_tensor(
        "vones", [128, 2 * (T // 128)], F32R, kind="ExternalInput"
    )
    y_d = nc.dram_tensor("y", [NTOK, C], F32, kind="ExternalOutput")

    with tile.TileContext(nc) as tc:
        with (
            tc.tile_pool(name="consts", bufs=1) as consts,
            tc.tile_pool(name="xs", bufs=2) as xs_pool,
            tc.tile_pool(name="acts", bufs=1) as acts,
            tc.tile_pool(name="big", bufs=2) as big,
            tc.tile_pool(name="tmp", bufs=2) as tmp_pool,
            tc.tile_pool(name="es", bufs=2) as es_pool,
            tc.tile_pool(name="rows", bufs=2) as rows,
            tc.tile_pool(name="ibc", bufs=2) as ibc_pool,
            tc.tile_pool(name="onorm", bufs=1) as on_pool,
            tc.tile_pool(name="ystage", bufs=4) as y_pool,
            tc.tile_pool(name="psA", bufs=2, space="PSUM") as psA,
            tc.tile_pool(name="psB", bufs=2, space="PSUM") as psB,
        ):
            # ---- constants ----
            wcat_sb = consts.tile([128, KC, 384], F32R, tag="wcat")
            for k in range(KC):
                nc.sync.dma_start(
                    out=wcat_sb[:, k, :], in_=wcat_d[128 * k : 128 * (k + 1), :]
                )
            bias_sb = consts.tile([128, 3], F32, tag="bias")
            nc.sync.dma_start(out=bias_sb, in_=bias_d[:, :])
            wo_sb = consts.tile([128, 2, C], F32R, tag="wo")
            for k in range(2):
                nc.sync.dma_start(
                    out=wo_sb[:, k, :], in_=wo_d[128 * k : 128 * (k + 1), :]
                )
            pmat_sb = consts.tile([128, 128], F32R, tag="pmat")
            nc.sync.dma_start(out=pmat_sb, in_=pmat_d[:, :])
            dmask_sb = consts.tile([128, 128], F32R, tag="dmask")
            nc.sync.dma_start(out=dmask_sb, in_=dmask_d[:, :])
            ident_sb = consts.tile([128, 64], F32R, tag="ident")
            nc.sync.dma_start(out=ident_sb, in_=ident_d[:, :])

            cos_sb = big.tile([128, NTOK], F32R, tag="big")
            nc.sync.dma_start(out=cos_sb, in_=cos_d[:, :])
            sin_sb = big.tile([128, NTOK], F32R, tag="big")
            nc.sync.dma_start(out=sin_sb, in_=sin_d[:, :])

            qT01 = acts.tile([128, NTOK], F32R, tag="qT01")
            qT23 = acts.tile([128, NTOK], F32R, tag="qT23")
            kvT = acts.tile([128, NTOK], F32R, tag="kvT")
            ktdup = acts.tile([128, NTOK], F32R, tag="ktdup")
            vaug = acts.tile([128, 2 * (T // 128), 65], F32R, tag="vaug")

            # ---- phase 1: QKV projection + RoPE + v-transpose, fused ----
            # Processing 1024-token groups keeps the PE stream dense: the
            # RoPE rotate-matmuls and v-transposes of group g interleave
            # with the QKV matmuls of group g+1, so the HAM never
            # re-throttles between phases.
            nc.sync.dma_start(
                out=vaug[:, :, 64:65], in_=vones_d[:, :].unsqueeze(2)
            )
            qkv_dst = [qT01, qT23, kvT]
            NJ = T // 128  # 16
            for ng in range(NTOK // 1024):
                base = 1024 * ng
                ps0 = psA.tile([128, 1024], F32, tag="a")
                ps1 = psA.tile([128, 1024], F32, tag="a")
                ps2 = psB.tile([128, 1024], F32, tag="o")
                pss = [ps0, ps1, ps2]
                for k in range(KC):
                    xt = xs_pool.tile([128, 1024], F32R, tag="xs")
                    nc.sync.dma_start(
                        out=xt,
                        in_=xT_d[128 * k : 128 * (k + 1), base : base + 1024],
                    )
                    for m in range(3):
                        for c2 in range(2):
                            nc.tensor.matmul(
                                pss[m][:, 512 * c2 : 512 * (c2 + 1)],
                                wcat_sb[:, k, 128 * m : 128 * (m + 1)],
                                xt[:, 512 * c2 : 512 * (c2 + 1)],
                                start=(k == 0),
                                stop=(k == KC - 1),
                            )
                for m in range(3):
                    nc.scalar.activation(
                        out=qkv_dst[m][:, base : base + 1024],
                        in_=pss[m],
                        func=AF.Identity,
                        bias=bias_sb[:, m : m + 1],
                        scale=1.0,
                    )
                # RoPE for this token group (token-pointwise)
                for dst, rn in ((qT01, 128), (qT23, 128), (kvT, 64)):
                    rot = psB.tile([128, 1024], F32, tag="o")
                    for c2 in range(2):
                        nc.tensor.matmul(
                            rot[:rn, 512 * c2 : 512 * (c2 + 1)],
                            pmat_sb[:rn, :rn],
                            dst[:rn, base + 512 * c2 : base + 512 * (c2 + 1)],
                            start=True,
                            stop=True,
                        )
                    tmp = tmp_pool.tile([128, 1024], F32, tag="tmp")
                    nc.vector.tensor_mul(
                        tmp[:rn], rot[:rn, :], sin_sb[:rn, base : base + 1024]
                    )
                    nc.vector.tensor_mul(
                        dst[:rn, base : base + 1024],
                        dst[:rn, base : base + 1024],
                        cos_sb[:rn, base : base + 1024],
                    )
                    nc.vector.tensor_add(
                        dst[:rn, base : base + 1024],
                        dst[:rn, base : base + 1024],
                        tmp[:rn],
                    )
                # v transposes for this token group (v is not roped)
                b2 = ng // 2
                for jj in range(8):
                    jt = (ng % 2) * 8 + jj
                    vps = psB.tile([128, 64], F32R, tag="o")
                    nc.tensor.transpose(
                        vps,
                        kvT[64:128, T * b2 + 128 * jt : T * b2 + 128 * (jt + 1)],
                        ident_sb[64:128, :],
                    )
                    nc.vector.tensor_copy(
                        vaug[:, b2 * NJ + jt, 0:64], vps
                    )

            # duplicate roped kT into partitions 64:128 for odd heads
            nc.sync.dma_start(out=ktdup[64:128, :], in_=kvT[0:64, :])

            # ---- phase 3: attention, per (batch, head, tq-half) ----
            OT0 = big.tile([128, NTOK], F32R, tag="big")
            OT1 = big.tile([128, NTOK], F32R, tag="big")
            OT = [OT0, OT1]
            for b2 in range(B):
                for hp in range(2):  # head pair (2hp, 2hp+1)
                    qtile = OTq = [qT01, qT23][hp]
                    for v2 in range(2):
                        tq0 = 1024 * v2
                        jmax = 8 * v2 + 8
                        ops_e = psB.tile([65, 1024], F32, tag="o")
                        ops_o = psB.tile([65, 1024], F32, tag="o")
                        last_bank = [8 * v2 + 3, jmax - 1]
                        for j in range(jmax):
                            tqs = max(128 * j, tq0)
                            W = tq0 + 1024 - tqs
                            ksl = slice(T * b2 + 128 * j, T * b2 + 128 * (j + 1))
                            sps_e = psA.tile([128, 1024], F32, tag="a")
                            sps_o = psA.tile([128, 1024], F32, tag="a")
                            off = 0
                            while off < W:
                                nw = min(512, W - off)
                                qsl = slice(T * b2 + tqs + off,
                                            T * b2 + tqs + off + nw)
                                # even head rows 0:64, odd head rows 64:128 —
                                # adjacent issue -> concurrent PE row groups
                                nc.tensor.matmul(
                                    sps_e[:, off : off + nw],
                                    kvT[0:64, ksl], qtile[0:64, qsl],
                                    start=True, stop=True,
                                )
                                nc.tensor.matmul(
                                    sps_o[:, off : off + nw],
                                    ktdup[64:128, ksl], qtile[64:128, qsl],
                                    start=True, stop=True,
                                )
                                off += nw
                            es_e = es_pool.tile([128, 1024], F32R, tag="ese")
                            es_o = es_pool.tile([128, 1024], F32R, tag="eso")
                            nc.scalar.activation(
                                out=es_e[:, :W], in_=sps_e[:, :W],
                                func=AF.Exp, scale=SCALE,
                            )
                            nc.scalar.activation(
                                out=es_o[:, :W], in_=sps_o[:, :W],
                                func=AF.Exp, scale=SCALE,
                            )
                            if 128 * j >= tq0:
                                nc.vector.tensor_mul(
                                    es_e[:, 0:128], es_e[:, 0:128], dmask_sb
                                )
                                nc.vector.tensor_mul(
                                    es_o[:, 0:128], es_o[:, 0:128], dmask_sb
                                )
                            off = 0
                            while off < W:
                                oc = tqs - tq0 + off
                                bank = oc // 512
                                nw = min(512 - oc % 512, W - off)
                                for ops, es in ((ops_e, es_e), (ops_o, es_o)):
                                    nc.tensor.matmul(
                                        ops[:, oc : oc + nw],
                                        vaug[:, b2 * NJ + j, :],
                                        es[:, off : off + nw],
                                        start=(j == 0),
                                        stop=(j == last_bank[bank]),
                                    )
                                off += nw
                        # softmax denominators: 1/rowsum via exp(-ln(.))
                        tok0 = T * b2 + tq0
                        for par, ops in ((0, ops_e), (1, ops_o)):
                            lnr = rows.tile([1, 1024], F32, tag="lnr")
                            nc.scalar.activation(
                                out=lnr, in_=ops[64:65, :], func=AF.Ln
                            )
                            nc.scalar.activation(
                                out=lnr, in_=lnr, func=AF.Exp, scale=-1.0
                            )
                            scr = nc.dram_tensor(
                                f"scr_{b2}_{hp}_{par}_{v2}", [1, 1024], F32
                            )
                            nc.sync.dma_start(out=scr[:, :], in_=lnr)
                            ibc = ibc_pool.tile([64, 1024], F32, tag="ibc")
                            nc.sync.dma_start(
                                out=ibc,
                                in_=scr[:, :].partition_broadcast(64).squeeze(1),
                            )
                            ot = OT[hp]
                            if par == 0:
                                nc.vector.tensor_mul(
                                    ot[0:64, tok0 : tok0 + 1024],
                                    ops[0:64, :],
                                    ibc,
                                )
                            else:
                                on = on_pool.tile([64, 1024], F32R, tag="on")
                                nc.vector.tensor_mul(on, ops[0:64, :], ibc)
                                nc.sync.dma_start(
                                    out=ot[64:128, tok0 : tok0 + 1024], in_=on
                                )

            # ---- phase 4: partial output projection ----
            for mt in range(NTOK // 128):
                for ncc in range(C // 512):
                    pool = psA if ncc % 2 == 0 else psB
                    tag = "a" if ncc % 2 == 0 else "o"
                    yp = pool.tile([128, 512], F32, tag=tag)
                    for kq in range(2):
                        nc.tensor.matmul(
                            yp,
                            OT[kq][:, 128 * mt : 128 * (mt + 1)],
                            wo_sb[:, kq, 512 * ncc : 512 * (ncc + 1)],
                            start=(kq == 0),
                            stop=(kq == 1),
                        )
                    ys = y_pool.tile([128, 512], F32, tag="ys")
                    if ncc % 2 == 0:
                        nc.scalar.copy(ys, yp)
                    else:
                        nc.vector.tensor_copy(ys, yp)
                    nc.sync.dma_start(
                        out=y_d[128 * mt : 128 * (mt + 1),
                                512 * ncc : 512 * (ncc + 1)],
                        in_=ys,
                    )
    _split_waits(nc)
    return nc


def _host_inputs(x, sinusoidal_pos, Wq, bq, Wk, bk, Wv, bv, Wo):
    xT = np.ascontiguousarray(x.reshape(NTOK, C).T).astype(np.float32)

    sp = np.asarray(sinusoidal_pos, dtype=np.float32).reshape(T, D)
    cosd = np.repeat(sp[:, 0::2], 2, axis=1)     # [T, D]
    sind = np.repeat(sp[:, 1::2], 2, axis=1)
    cosb = np.tile(cosd.T, (1, B))               # [D, NTOK]
    sinb = np.tile(sind.T, (1, B))
    cos2 = np.ascontiguousarray(np.concatenate([cosb, cosb], 0))  # [128, NTOK]
    sin2 = np.ascontiguousarray(np.concatenate([sinb, sinb], 0))

    P = np.zeros((D, D), dtype=np.float32)
    P[: D // 2, D // 2 :] = np.eye(D // 2)
    P[D // 2 :, : D // 2] = -np.eye(D // 2)
    pmat = np.zeros((128, 128), dtype=np.float32)
    pmat[:64, :64] = P
    pmat[64:, 64:] = P

    f = np.arange(128)[None, :]
    p = np.arange(128)[:, None]
    dmask = (f >= p).astype(np.float32)          # S^T diag block: keep tk<=tq

    ident2 = np.concatenate([np.eye(64), np.eye(64)], 0).astype(np.float32)

    shared = {
        "xT": xT, "cos2": cos2, "sin2": sin2,
        "pmat": pmat, "dmask": dmask, "ident2": ident2,
        "vones": np.ones((128, 2 * (T // 128)), dtype=np.float32),
    }
    per_core = []
    for c in range(8):
        # q head h uses kv head h % KVH (jnp.tile), so core c owns
        # q heads {c, c+8, c+16, c+24} and kv head c.
        heads = [c + KVH * g for g in range(HPC)]
        qrows = np.concatenate([np.arange(D * h, D * (h + 1)) for h in heads])
        Wq_c = Wq[qrows]
        Wk_c = Wk[D * c : D * (c + 1)]
        Wv_c = Wv[D * c : D * (c + 1)]
        wcatT = np.ascontiguousarray(
            np.concatenate([Wq_c, Wk_c, Wv_c], 0).T
        ).astype(np.float32)
        bcat = np.concatenate(
            [bq[qrows], bk[D * c : D * (c + 1)], bv[D * c : D * (c + 1)]]
        ).astype(np.float32)
        bqkv = np.ascontiguousarray(bcat.reshape(3, 128).T)
        woR = np.ascontiguousarray(Wo[:, qrows].T).astype(np.float32)
        per_core.append(dict(shared, wcatT=wcatT, bqkv=bqkv, woR=woR))
    return per_core


def kernel(x, mask, sinusoidal_pos, Wq, bq, Wk, bk, Wv, bv, Wo, bo):
    x = np.asarray(x, dtype=np.float32)
    in_maps = _host_inputs(
        x, sinusoidal_pos,
        np.asarray(Wq, np.float32), np.asarray(bq, np.float32),
        np.asarray(Wk, np.float32), np.asarray(bk, np.float32),
        np.asarray(Wv, np.float32), np.asarray(bv, np.float32),
        np.asarray(Wo, np.float32),
    )
    if "nc" not in _NC_CACHE:
        _NC_CACHE["nc"] = build_nc()
    res = run_bass_kernel_spmd(
        _NC_CACHE["nc"], in_maps, core_ids=list(range(8))
    )
    y = np.zeros((NTOK, C), dtype=np.float32)
    for r in res.results:
        y += r["y"]
    y += np.asarray(bo, np.float32)[None, :]
    return y.reshape(B, T, C)


# revision 20
# speedup vs baseline: 1.4809x; 1.0218x over previous
"""Trainium2 Bass kernel for GQA multi-head attention with RoPE.

Sharding: tensor-parallel over heads. Core c owns q-heads 4c..4c+3 and
kv-head c. Each core computes its QKV projection slice, RoPE, causal
attention for its heads, and a partial output projection
(attn_out_local @ Wo[:, local].T). The host sums the 8 partial y's.

Device layouts (per core):
  xT      [C, B*T]   x transposed (replicated)
  wcatT   [C, 384]   [Wq_loc | Wk_loc | Wv_loc].T
  qT/kT   [d, tok]   head-dim on partitions ("T-layout")
  scores  S^T [tk partitions, tq free] so no transposes are needed:
          exp(S^T) feeds P@V directly as the moving operand with
          v_aug = [v | ones] stationary; the ones row yields softmax
          row-sums in partition 64 of the PV accumulator.
  y       [B*T, C]   partial; host adds the 8 partials + bo.

Matmul operands are float32r (fp32 with 11-bit mantissa, full PE rate
at N>=256); the QKV projection inputs (x, Wqkv) are fp16 (10-bit
mantissa, halves the HBM-bound x traffic). PSUM accumulation is fp32.
"""

import sys

sys.path.insert(0, "/opt/trn_rl_repo")

import numpy as np

import bass_rust
import concourse.bass as bass
import concourse.tile as tile
from concourse import mybir
from concourse.bass_utils import run_bass_kernel_spmd

B, T, C = 2, 2048, 2048
H, KVH, D = 32, 8, 64
NTOK = B * T                 # 4096
HPC = H // 8                 # 4 q heads per core
QL = HPC * D                 # 256 local q dims
KC = C // 128                # 16 contraction chunks
SCALE = float(D) ** -0.5

F32 = mybir.dt.float32
F32R = mybir.dt.float32r
F16 = mybir.dt.float16
AF = mybir.ActivationFunctionType

_NC_CACHE = {}


def _split_waits(nc, limit=1):
    """Walrus in this toolchain allows only one sync-wait per instruction.

    Tile emits instructions with several sem waits (drain/barrier, phase
    boundaries). Hoist the excess onto same-engine NoOps inserted right
    before the instruction — program order on the engine queue preserves
    the wait semantics.
    """
    ctr = 0
    for f in nc.m.functions:
        for blk in f.blocks:
            out = []
            changed = False
            for inst in list(blk.instructions):
                si = inst.sync_info
                if si is not None and len(si.on_wait) > limit:
                    waits = list(si.on_wait)
                    keep, excess = waits[:limit], waits[limit:]
                    for i in range(0, len(excess), limit):
                        ctr += 1
                        nop = mybir.InstNoOp(
                            name=f"I-wsplit-{ctr}", ins=[], outs=[]
                        )
                        nop.engine = inst.engine
                        nop.sync_info = bass_rust.SyncInfo(
                            on_wait=excess[i : i + limit], on_update=[]
                        )
                        out.append(nop)
                        changed = True
                    inst.sync_info = bass_rust.SyncInfo(
                        on_wait=keep, on_update=list(si.on_update)
                    )
                out.append(inst)
            if changed:
                blk.instructions = out
    return ctr


def build_nc():
    nc = bass.Bass(trn_type="TRN2")

    xT_d = nc.dram_tensor("xT", [C, NTOK], F16, kind="ExternalInput")
    wcat_d = nc.dram_tensor("wcatT", [C, 384], F16, kind="ExternalInput")
    bias_d = nc.dram_tensor("bqkv", [128, 3], F32, kind="ExternalInput")
    wo_d = nc.dram_tensor("woR", [QL, C], F32R, kind="ExternalInput")
    cos_d = nc.dram_tensor("cos2", [128, NTOK], F32R, kind="ExternalInput")
    sin_d = nc.dram_tensor("sin2", [128, NTOK], F32R, kind="ExternalInput")
    pmat_d = nc.dram_tensor("pmat", [128, 128], F32R, kind="ExternalInput")
    dmask_d = nc.dram_tensor("dmask", [128, 128], F32R, kind="ExternalInput")
    ident_d = nc.dram_tensor("ident2", [128, 64], F32R, kind="ExternalInput")
    vones_d = nc.dram_tensor(
        "vones", [128, 2 * (T // 128)], F32R, kind="ExternalInput"
    )
    y_d = nc.dram_tensor("y", [NTOK, C], F32, kind="ExternalOutput")

    with tile.TileContext(nc) as tc:
        with (
            tc.tile_pool(name="consts", bufs=1) as consts,
            tc.tile_pool(name="xs", bufs=4) as xs_pool,
            tc.tile_pool(name="acts", bufs=1) as acts,
            tc.tile_pool(name="big", bufs=2) as big,
            tc.tile_pool(name="tmp", bufs=2) as tmp_pool,
            tc.tile_pool(name="es", bufs=2) as es_pool,
            tc.tile_pool(name="rows", bufs=2) as rows,
            tc.tile_pool(name="ibc", bufs=2) as ibc_pool,
            tc.tile_pool(name="onorm", bufs=1) as on_pool,
            tc.tile_pool(name="ystage", bufs=4) as y_pool,
            tc.tile_pool(name="psA", bufs=2, space="PSUM") as psA,
            tc.tile_pool(name="psB", bufs=2, space="PSUM") as psB,
        ):
            # ---- constants ----
            wcat_sb = consts.tile([128, KC, 384], F16, tag="wcat")
            for k in range(KC):
                nc.sync.dma_start(
                    out=wcat_sb[:, k, :], in_=wcat_d[128 * k : 128 * (k + 1), :]
                )
            bias_sb = consts.tile([128, 3], F32, tag="bias")
            nc.sync.dma_start(out=bias_sb, in_=bias_d[:, :])
            wo_sb = consts.tile([128, 2, C], F32R, tag="wo")
            for k in range(2):
                nc.sync.dma_start(
                    out=wo_sb[:, k, :], in_=wo_d[128 * k : 128 * (k + 1), :]
                )
            pmat_sb = consts.tile([128, 128], F32R, tag="pmat")
            nc.sync.dma_start(out=pmat_sb, in_=pmat_d[:, :])
            dmask_sb = consts.tile([128, 128], F32R, tag="dmask")
            nc.sync.dma_start(out=dmask_sb, in_=dmask_d[:, :])
            ident_sb = consts.tile([128, 64], F32R, tag="ident")
            nc.sync.dma_start(out=ident_sb, in_=ident_d[:, :])

            cos_sb = big.tile([128, NTOK], F32R, tag="big")
            nc.sync.dma_start(out=cos_sb, in_=cos_d[:, :])
            sin_sb = big.tile([128, NTOK], F32R, tag="big")
            nc.sync.dma_start(out=sin_sb, in_=sin_d[:, :])

            qT01 = acts.tile([128, NTOK], F32R, tag="qT01")
            qT23 = acts.tile([128, NTOK], F32R, tag="qT23")
            kvT = acts.tile([128, NTOK], F32R, tag="kvT")
            ktdup = acts.tile([128, NTOK], F32R, tag="ktdup")
            vaug = acts.tile([128, 2 * (T // 128), 65], F32R, tag="vaug")

            # ---- phase 1: QKV projection + RoPE + v-transpose, fused ----
            # Processing 1024-token groups keeps the PE stream dense: the
            # RoPE rotate-matmuls and v-transposes of group g interleave
            # with the QKV matmuls of group g+1, so the HAM never
            # re-throttles between phases.
            nc.sync.dma_start(
                out=vaug[:, :, 64:65], in_=vones_d[:, :].unsqueeze(2)
            )
            qkv_dst = [qT01, qT23, kvT]
            NJ = T // 128  # 16
            for ng in range(NTOK // 1024):
                base = 1024 * ng
                ps0 = psA.tile([128, 1024], F32, tag="a")
                ps1 = psA.tile([128, 1024], F32, tag="a")
                ps2 = psB.tile([128, 1024], F32, tag="o")
                pss = [ps0, ps1, ps2]
                for k in range(KC):
                    xt = xs_pool.tile([128, 1024], F16, tag="xs")
                    nc.sync.dma_start(
                        out=xt,
                        in_=xT_d[128 * k : 128 * (k + 1), base : base + 1024],
                    )
                    for m in range(3):
                        for c2 in range(2):
                            nc.tensor.matmul(
                                pss[m][:, 512 * c2 : 512 * (c2 + 1)],
                                wcat_sb[:, k, 128 * m : 128 * (m + 1)],
                                xt[:, 512 * c2 : 512 * (c2 + 1)],
                                start=(k == 0),
                                stop=(k == KC - 1),
                            )
                for m in range(3):
                    nc.scalar.activation(
                        out=qkv_dst[m][:, base : base + 1024],
                        in_=pss[m],
                        func=AF.Identity,
                        bias=bias_sb[:, m : m + 1],
                        scale=1.0,
                    )
                # RoPE for this token group (token-pointwise)
                for dst, rn in ((qT01, 128), (qT23, 128), (kvT, 64)):
                    rot = psB.tile([128, 1024], F32, tag="o")
                    for c2 in range(2):
                        nc.tensor.matmul(
                            rot[:rn, 512 * c2 : 512 * (c2 + 1)],
                            pmat_sb[:rn, :rn],
                            dst[:rn, base + 512 * c2 : base + 512 * (c2 + 1)],
                            start=True,
                            stop=True,
                        )
                    tmp = tmp_pool.tile([128, 1024], F32, tag="tmp")
                    nc.vector.tensor_mul(
                        tmp[:rn], rot[:rn, :], sin_sb[:rn, base : base + 1024]
                    )
                    nc.vector.tensor_mul(
                        dst[:rn, base : base + 1024],
                        dst[:rn, base : base + 1024],
                        cos_sb[:rn, base : base + 1024],
                    )
                    nc.vector.tensor_add(
                        dst[:rn, base : base + 1024],
                        dst[:rn, base : base + 1024],
                        tmp[:rn],
                    )
                # v transposes for this token group (v is not roped)
                b2 = ng // 2
                for jj in range(8):
                    jt = (ng % 2) * 8 + jj
                    vps = psB.tile([128, 64], F32R, tag="o")
                    nc.tensor.transpose(
                        vps,
                        kvT[64:128, T * b2 + 128 * jt : T * b2 + 128 * (jt + 1)],
                        ident_sb[64:128, :],
                    )
                    nc.vector.tensor_copy(
                        vaug[:, b2 * NJ + jt, 0:64], vps
                    )

            # duplicate roped kT into partitions 64:128 for odd heads
            nc.sync.dma_start(out=ktdup[64:128, :], in_=kvT[0:64, :])

            # ---- phase 3: attention, per (batch, head, tq-half) ----
            OT0 = big.tile([128, NTOK], F32R, tag="big")
            OT1 = big.tile([128, NTOK], F32R, tag="big")
            OT = [OT0, OT1]
            for b2 in range(B):
                for hp in range(2):  # head pair (2hp, 2hp+1)
                    qtile = OTq = [qT01, qT23][hp]
                    for v2 in range(2):
                        tq0 = 1024 * v2
                        jmax = 8 * v2 + 8
                        ops_e = psB.tile([65, 1024], F32, tag="o")
                        ops_o = psB.tile([65, 1024], F32, tag="o")
                        last_bank = [8 * v2 + 3, jmax - 1]
                        for j in range(jmax):
                            tqs = max(128 * j, tq0)
                            W = tq0 + 1024 - tqs
                            ksl = slice(T * b2 + 128 * j, T * b2 + 128 * (j + 1))
                            sps_e = psA.tile([128, 1024], F32, tag="a")
                            sps_o = psA.tile([128, 1024], F32, tag="a")
                            off = 0
                            while off < W:
                                nw = min(512, W - off)
                                qsl = slice(T * b2 + tqs + off,
                                            T * b2 + tqs + off + nw)
                                # even head rows 0:64, odd head rows 64:128 —
                                # adjacent issue -> concurrent PE row groups
                                nc.tensor.matmul(
                                    sps_e[:, off : off + nw],
                                    kvT[0:64, ksl], qtile[0:64, qsl],
                                    start=True, stop=True,
                                )
                                nc.tensor.matmul(
                                    sps_o[:, off : off + nw],
                                    ktdup[64:128, ksl], qtile[64:128, qsl],
                                    start=True, stop=True,
                                )
                                off += nw
                            es_e = es_pool.tile([128, 1024], F32R, tag="ese")
                            es_o = es_pool.tile([128, 1024], F32R, tag="eso")
                            nc.scalar.activation(
                                out=es_e[:, :W], in_=sps_e[:, :W],
                                func=AF.Exp, scale=SCALE,
                            )
                            nc.scalar.activation(
                                out=es_o[:, :W], in_=sps_o[:, :W],
                                func=AF.Exp, scale=SCALE,
                            )
                            if 128 * j >= tq0:
                                nc.vector.tensor_mul(
                                    es_e[:, 0:128], es_e[:, 0:128], dmask_sb
                                )
                                nc.vector.tensor_mul(
                                    es_o[:, 0:128], es_o[:, 0:128], dmask_sb
                                )
                            off = 0
                            while off < W:
                                oc = tqs - tq0 + off
                                bank = oc // 512
                                nw = min(512 - oc % 512, W - off)
                                for ops, es in ((ops_e, es_e), (ops_o, es_o)):
                                    nc.tensor.matmul(
                                        ops[:, oc : oc + nw],
                                        vaug[:, b2 * NJ + j, :],
                                        es[:, off : off + nw],
                                        start=(j == 0),
                                        stop=(j == last_bank[bank]),
                                    )
                                off += nw
                        # softmax denominators: 1/rowsum via exp(-ln(.))
                        tok0 = T * b2 + tq0
                        for par, ops in ((0, ops_e), (1, ops_o)):
                            lnr = rows.tile([1, 1024], F32, tag="lnr")
                            nc.scalar.activation(
                                out=lnr, in_=ops[64:65, :], func=AF.Ln
                            )
                            nc.scalar.activation(
                                out=lnr, in_=lnr, func=AF.Exp, scale=-1.0
                            )
                            scr = nc.dram_tensor(
                                f"scr_{b2}_{hp}_{par}_{v2}", [1, 1024], F32
                            )
                            nc.sync.dma_start(out=scr[:, :], in_=lnr)
                            ibc = ibc_pool.tile([64, 1024], F32, tag="ibc")
                            nc.sync.dma_start(
                                out=ibc,
                                in_=scr[:, :].partition_broadcast(64).squeeze(1),
                            )
                            ot = OT[hp]
                            if par == 0:
                                nc.vector.tensor_mul(
                                    ot[0:64, tok0 : tok0 + 1024],
                                    ops[0:64, :],
                                    ibc,
                                )
                            else:
                                on = on_pool.tile([64, 1024], F32R, tag="on")
                                nc.vector.tensor_mul(on, ops[0:64, :], ibc)
                                nc.sync.dma_start(
                                    out=ot[64:128, tok0 : tok0 + 1024], in_=on
                                )

            # ---- phase 4: partial output projection ----
            for mt in range(NTOK // 128):
                for ncc in range(C // 512):
                    pool = psA if ncc % 2 == 0 else psB
                    tag = "a" if ncc % 2 == 0 else "o"
                    yp = pool.tile([128, 512], F32, tag=tag)
                    for kq in range(2):
                        nc.tensor.matmul(
                            yp,
                            OT[kq][:, 128 * mt : 128 * (mt + 1)],
                            wo_sb[:, kq, 512 * ncc : 512 * (ncc + 1)],
                            start=(kq == 0),
                            stop=(kq == 1),
                        )
                    ys = y_pool.tile([128, 512], F32, tag="ys")
                    if ncc % 2 == 0:
                        nc.scalar.copy(ys, yp)
                    else:
                        nc.vector.tensor_copy(ys, yp)
                    nc.sync.dma_start(
                        out=y_d[128 * mt : 128 * (mt + 1),
                                512 * ncc : 512 * (ncc + 1)],
                        in_=ys,
                    )
    _split_waits(nc)
    return nc


def _host_inputs(x, sinusoidal_pos, Wq, bq, Wk, bk, Wv, bv, Wo):
    xT = np.ascontiguousarray(x.reshape(NTOK, C).T).astype(np.float16)

    sp = np.asarray(sinusoidal_pos, dtype=np.float32).reshape(T, D)
    cosd = np.repeat(sp[:, 0::2], 2, axis=1)     # [T, D]
    sind = np.repeat(sp[:, 1::2], 2, axis=1)
    cosb = np.tile(cosd.T, (1, B))               # [D, NTOK]
    sinb = np.tile(sind.T, (1, B))
    cos2 = np.ascontiguousarray(np.concatenate([cosb, cosb], 0))  # [128, NTOK]
    sin2 = np.ascontiguousarray(np.concatenate([sinb, sinb], 0))

    P = np.zeros((D, D), dtype=np.float32)
    P[: D // 2, D // 2 :] = np.eye(D // 2)
    P[D // 2 :, : D // 2] = -np.eye(D // 2)
    pmat = np.zeros((128, 128), dtype=np.float32)
    pmat[:64, :64] = P
    pmat[64:, 64:] = P

    f = np.arange(128)[None, :]
    p = np.arange(128)[:, None]
    dmask = (f >= p).astype(np.float32)          # S^T diag block: keep tk<=tq

    ident2 = np.concatenate([np.eye(64), np.eye(64)], 0).astype(np.float32)

    shared = {
        "xT": xT, "cos2": cos2, "sin2": sin2,
        "pmat": pmat, "dmask": dmask, "ident2": ident2,
        "vones": np.ones((128, 2 * (T // 128)), dtype=np.float32),
    }
    per_core = []
    for c in range(8):
        # q head h uses kv head h % KVH (jnp.tile), so core c owns
        # q heads {c, c+8, c+16, c+24} and kv head c.
        heads = [c + KVH * g for g in range(HPC)]
        qrows = np.concatenate([np.arange(D * h, D * (h + 1)) for h in heads])
        Wq_c = Wq[qrows]
        Wk_c = Wk[D * c : D * (c + 1)]
        Wv_c = Wv[D * c : D * (c + 1)]
        wcatT = np.ascontiguousarray(
            np.concatenate([Wq_c, Wk_c, Wv_c], 0).T
        ).astype(np.float16)
        bcat = np.concatenate(
            [bq[qrows], bk[D * c : D * (c + 1)], bv[D * c : D * (c + 1)]]
        ).astype(np.float32)
        bqkv = np.ascontiguousarray(bcat.reshape(3, 128).T)
        woR = np.ascontiguousarray(Wo[:, qrows].T).astype(np.float32)
        per_core.append(dict(shared, wcatT=wcatT, bqkv=bqkv, woR=woR))
    return per_core


def kernel(x, mask, sinusoidal_pos, Wq, bq, Wk, bk, Wv, bv, Wo, bo):
    x = np.asarray(x, dtype=np.float32)
    in_maps = _host_inputs(
        x, sinusoidal_pos,
        np.asarray(Wq, np.float32), np.asarray(bq, np.float32),
        np.asarray(Wk, np.float32), np.asarray(bk, np.float32),
        np.asarray(Wv, np.float32), np.asarray(bv, np.float32),
        np.asarray(Wo, np.float32),
    )
    if "nc" not in _NC_CACHE:
        _NC_CACHE["nc"] = build_nc()
    res = run_bass_kernel_spmd(
        _NC_CACHE["nc"], in_maps, core_ids=list(range(8))
    )
    y = np.zeros((NTOK, C), dtype=np.float32)
    for r in res.results:
        y += r["y"]
    y += np.asarray(bo, np.float32)[None, :]
    return y.reshape(B, T, C)
